# revision 8
# baseline (speedup 1.0000x reference)
"""Trainium2 Bass kernel for additive (Bahdanau) attention.

Computes, for query [B, D], keys [B, S, D], Wq/Wk [H, D], w_att [H]:
    q      = query @ Wq.T                    [B, H]
    k      = einsum('bsd,hd->bsh', keys, Wk) [B, S, H]
    comb   = tanh(q[:, None, :] + k)         [B, S, H]
    scores = einsum('bsh,h->bs', comb, w_att)
    attn   = softmax(scores, -1)
    out    = einsum('bs,bsd->bd', attn, keys)
Returns (out, attn).

Strategy: pure data parallel over batch — 8 NeuronCores, 4 batches each;
weights replicated.  Per core, keys stream in as bf16 (cast during the
SWDGE DMA), get transposed on-chip with one batched DMA-xbar call per
batch (d onto partitions) for the Wk matmul; tanh is fused with the
per-partition q bias on the scalar engine; scores use w_att as a
1-column stationary matmul; softmax is computed unnormalized (exp with
accumulated denominator, deferred division), so the attn@keys matmul
streams per s-tile without a global barrier.
"""

import sys

sys.path.insert(0, "/opt/trn_rl_repo")

import numpy as np

import concourse.bacc as bacc
import concourse.tile as tile
from concourse import mybir
from concourse import bass_utils

B, S, D, H = 32, 2048, 512, 512
NCORES = 8
BC = B // NCORES  # batches per core
P = 128
ST = S // P  # s-tiles per batch (16)
SB = 4  # s-blocks per batch (512 cols each)
HC = H // P  # h chunks (4)
DC = D // P  # d chunks (4)
F32 = mybir.dt.float32
BF16 = mybir.dt.bfloat16
TANH = mybir.ActivationFunctionType.Tanh
EXP = mybir.ActivationFunctionType.Exp


def _body(ctx, tc, nc, q_d, k_d, wq_d, wk_d, wa_d, out_d, attn_d):
    singles = ctx.enter_context(tc.tile_pool(name="singles", bufs=1))
    keys_pool = ctx.enter_context(tc.tile_pool(name="keys", bufs=4))
    keysT_pool = ctx.enter_context(tc.tile_pool(name="keysT", bufs=3))

    # kick off all keys loads first: they are the long pole and must not
    # queue behind the small weight-setup DMAs
    keys_tiles = []
    for b in range(BC):
        keys_sb = keys_pool.tile([P, ST, D], BF16, tag="keys")
        nc.gpsimd.dma_start(
            out=keys_sb, in_=k_d.ap()[b].rearrange("(st p) d -> p st d", p=P)
        )
        keys_tiles.append(keys_sb)

    # --- one-time setup: weights ---------------------------------------
    # natural-layout bf16 loads: [p, chunk, d] with p = h within chunk
    wk_nat = singles.tile([P, HC, D], BF16)
    nc.gpsimd.dma_start(out=wk_nat, in_=wk_d.ap().rearrange("(hc p) d -> p hc d", p=P))
    wq_nat = singles.tile([P, HC, D], BF16)
    nc.gpsimd.dma_start(out=wq_nat, in_=wq_d.ap().rearrange("(hc p) d -> p hc d", p=P))

    # block-transposed weights via one xbar call each:
    # wkT[p, j, l] = wk_nat[l, j*128+p]  ->  j = hc*DC + dc encodes (hc, dc);
    # lhsT chunk for (dc, hc) is wkT[:, hc*DC+dc, :] = WkT[d-chunk, h-chunk].
    wkT = singles.tile([P, HC * DC, P], BF16)
    nc.sync.dma_start(out=wkT, in_=wk_nat.rearrange("p hc d -> p (hc d)"), transpose=True)
    wqT = singles.tile([P, HC * DC, P], BF16)
    nc.sync.dma_start(out=wqT, in_=wq_nat.rearrange("p hc d -> p (hc d)"), transpose=True)

    # w_att as per-partition columns [p=h-in-chunk, hc]
    wa_col = singles.tile([P, HC], BF16)
    nc.gpsimd.dma_start(out=wa_col, in_=wa_d.ap().rearrange("(hc p) -> p hc", p=P))

    # query transposed [p=d-in-chunk, dc, b]
    qT = singles.tile([P, DC, BC], BF16)
    for dc in range(DC):
        nc.gpsimd.dma_start(
            out=qT[:, dc, :],
            in_=q_d.ap()[:, dc * P : (dc + 1) * P].rearrange("b p -> p b"),
        )

    # identity scalar for PE vector transposes
    ones11 = singles.tile([1, 1], F32)
    nc.vector.memset(ones11, 1.0)

    # Q = query @ Wq.T laid out [p=h-in-chunk, hc, b] (f32, used as ACT bias)
    q_sb = singles.tile([P, HC, BC], F32)
    with tc.tile_pool(name="ps_setup", bufs=2, space="PSUM") as psq:
        for hc in range(HC):
            pq = psq.tile([P, BC], F32)
            for dc in range(DC):
                nc.tensor.matmul(
                    pq,
                    lhsT=wqT[:, hc * DC + dc, :],
                    rhs=qT[:, dc, :],
                    start=(dc == 0),
                    stop=(dc == DC - 1),
                )
            nc.vector.tensor_copy(q_sb[:, hc, :], pq)

    # --- main loop pools ------------------------------------------------
    comb_pool = ctx.enter_context(tc.tile_pool(name="comb", bufs=6))
    p_pool = ctx.enter_context(tc.tile_pool(name="prow", bufs=2))
    small_pool = ctx.enter_context(tc.tile_pool(name="small", bufs=8))
    outbuf_pool = ctx.enter_context(tc.tile_pool(name="outbuf", bufs=2))
    ps_k = ctx.enter_context(tc.tile_pool(name="ps_k", bufs=2, space="PSUM"))  # 2x2 banks
    ps_sc = ctx.enter_context(tc.tile_pool(name="ps_sc", bufs=1, space="PSUM"))  # 2 banks
    ps_pcol = ctx.enter_context(tc.tile_pool(name="ps_pcol", bufs=1, space="PSUM"))
    ps_out = ctx.enter_context(tc.tile_pool(name="ps_out", bufs=1, space="PSUM"))

    for b in range(BC):
        keys_sb = keys_tiles[b]
        # one batched xbar transpose: keysT[p, j, l] = keys_sb_flat[l, j*128+p]
        # with j = st*DC + dc; so [d-chunk dc | s-tile st] lives at mid j.
        keysT = keysT_pool.tile([P, ST * DC, P], BF16)
        nc.sync.dma_start(
            out=keysT, in_=keys_sb.rearrange("p st d -> p (st d)"), transpose=True
        )
        # view for the main matmul rhs: [p, dc, st, l]
        keysT_v = keysT.rearrange("p (st dc) l -> p dc st l", dc=DC)

        p_row = p_pool.tile([1, S], F32)
        denoms = small_pool.tile([1, 2], F32)
        pcol_bf = small_pool.tile([P, ST], BF16)
        ps_o = ps_out.tile([1, D], F32)

        for half in range(2):  # two s-block pairs of 1024 columns each
            # k-chunk matmuls + fused bias-tanh on [128, 1024] tiles
            comb_tiles = []
            for hc in range(HC):
                pk = ps_k.tile([P, 1024], F32)
                for sbi in range(2):
                    sb = half * 2 + sbi
                    for dc in range(DC):
                        nc.tensor.matmul(
                            pk[:, sbi * 512 : (sbi + 1) * 512],
                            lhsT=wkT[:, hc * DC + dc, :],
                            rhs=keysT_v[:, dc, sb * SB : (sb + 1) * SB, :],
                            start=(dc == 0),
                            stop=(dc == DC - 1),
                        )
                comb = comb_pool.tile([P, 1024], BF16)
                nc.scalar.activation(
                    comb, pk, TANH, bias=q_sb[:, hc, b : b + 1], scale=1.0
                )
                comb_tiles.append(comb)

            # scores[1, 1024] = sum_h w_att[h] * comb[h, s]
            psc = ps_sc.tile([1, 1024], F32)
            for sbi in range(2):
                for hc in range(HC):
                    nc.tensor.matmul(
                        psc[:, sbi * 512 : (sbi + 1) * 512],
                        lhsT=wa_col[:, hc : hc + 1],
                        rhs=comb_tiles[hc][:, sbi * 512 : (sbi + 1) * 512],
                        start=(hc == 0),
                        stop=(hc == HC - 1),
                    )
            # p = exp(scores) for 1024 cols, plus partial denominator
            hcol = slice(half * 1024, (half + 1) * 1024)
            nc.scalar.activation(
                p_row[:, hcol], psc, EXP, accum_out=denoms[:, half : half + 1]
            )
            # p columns for the attn@keys matmul: PE transpose of p chunks
            ppc = ps_pcol.tile([P, 8], F32)
            for j in range(8):
                st = half * 8 + j
                nc.tensor.transpose(
                    ppc[:, j : j + 1], p_row[0:1, st * P : (st + 1) * P], ones11
                )
            nc.vector.tensor_copy(pcol_bf[:, half * 8 : (half + 1) * 8], ppc)

            # out_un[1, d] += p[st].T @ keys[st]
            for j in range(8):
                st = half * 8 + j
                nc.tensor.matmul(
                    ps_o,
                    lhsT=pcol_bf[:, st : st + 1],
                    rhs=keys_sb[:, st, :],
                    start=(st == 0),
                    stop=(st == ST - 1),
                )

        # denominator + normalization
        dsum = small_pool.tile([1, 1], F32)
        nc.vector.tensor_reduce(
            dsum, denoms, axis=mybir.AxisListType.X, op=mybir.AluOpType.add
        )
        recip = small_pool.tile([1, 1], F32)
        nc.vector.reciprocal(recip, dsum)

        outrow = outbuf_pool.tile([1, D], F32)
        nc.vector.tensor_scalar_mul(outrow, ps_o, recip)
        nc.sync.dma_start(out=out_d.ap()[b : b + 1, :], in_=outrow)

        attn_row = outbuf_pool.tile([1, S], F32)
        nc.vector.tensor_scalar_mul(attn_row, p_row, recip)
        nc.sync.dma_start(out=attn_d.ap()[b : b + 1, :], in_=attn_row)


def _build():
    from contextlib import ExitStack

    nc = bacc.Bacc("TRN2", target_bir_lowering=False, debug=False, num_devices=NCORES)
    q_d = nc.dram_tensor("query", (BC, D), F32, kind="ExternalInput")
    k_d = nc.dram_tensor("keys", (BC, S, D), F32, kind="ExternalInput")
    wq_d = nc.dram_tensor("wq", (H, D), F32, kind="ExternalInput")
    wk_d = nc.dram_tensor("wk", (H, D), F32, kind="ExternalInput")
    wa_d = nc.dram_tensor("watt", (H,), F32, kind="ExternalInput")
    out_d = nc.dram_tensor("out", (BC, D), F32, kind="ExternalOutput")
    attn_d = nc.dram_tensor("attn", (BC, S), F32, kind="ExternalOutput")

    with tile.TileContext(nc) as tc:
        with ExitStack() as ctx:
            _body(ctx, tc, nc, q_d, k_d, wq_d, wk_d, wa_d, out_d, attn_d)
    nc.compile()
    return nc


_NC_CACHE = None


def _get_nc():
    global _NC_CACHE
    if _NC_CACHE is None:
        _NC_CACHE = _build()
    return _NC_CACHE


def kernel(query, keys, Wq, Wk, w_att, _trace=False, _tmpdir=None):
    nc = _get_nc()
    query = np.ascontiguousarray(query, dtype=np.float32)
    keys = np.ascontiguousarray(keys, dtype=np.float32)
    Wq = np.ascontiguousarray(Wq, dtype=np.float32)
    Wk = np.ascontiguousarray(Wk, dtype=np.float32)
    w_att = np.ascontiguousarray(w_att, dtype=np.float32)

    in_maps = [
        {
            "query": query[c * BC : (c + 1) * BC],
            "keys": keys[c * BC : (c + 1) * BC],
            "wq": Wq,
            "wk": Wk,
            "watt": w_att,
        }
        for c in range(NCORES)
    ]
    res = bass_utils.run_bass_kernel_spmd(
        nc, in_maps, core_ids=list(range(NCORES)), trace=_trace, tmpdir=_tmpdir
    )
    out = np.concatenate([res.results[c]["out"] for c in range(NCORES)], axis=0)
    attn = np.concatenate([res.results[c]["attn"] for c in range(NCORES)], axis=0)
    if _trace:
        return (out, attn), res
    return out, attn


# revision 9
# speedup vs baseline: 1.2576x; 1.2576x over previous
"""Trainium2 Bass kernel for additive (Bahdanau) attention.

Computes, for query [B, D], keys [B, S, D], Wq/Wk [H, D], w_att [H]:
    q      = query @ Wq.T                    [B, H]
    k      = einsum('bsd,hd->bsh', keys, Wk) [B, S, H]
    comb   = tanh(q[:, None, :] + k)         [B, S, H]
    scores = einsum('bsh,h->bs', comb, w_att)
    attn   = softmax(scores, -1)
    out    = einsum('bs,bsd->bd', attn, keys)
Returns (out, attn).

Strategy: pure data parallel over batch — 8 NeuronCores, 4 batches each;
weights replicated.  Per core, keys stream in as bf16 (cast during the
SWDGE DMA), get transposed on-chip with one batched DMA-xbar call per
batch (d onto partitions) for the Wk matmul; tanh is fused with the
per-partition q bias on the scalar engine; scores use w_att as a
1-column stationary matmul; softmax is computed unnormalized (exp with
accumulated denominator, deferred division), so the attn@keys matmul
streams per s-tile without a global barrier.
"""

import sys

sys.path.insert(0, "/opt/trn_rl_repo")

import numpy as np

import concourse.bacc as bacc
import concourse.tile as tile
from concourse import mybir
from concourse import bass_utils

B, S, D, H = 32, 2048, 512, 512
NCORES = 8
BC = B // NCORES  # batches per core
P = 128
ST = S // P  # s-tiles per batch (16)
SB = 4  # s-blocks per batch (512 cols each)
HC = H // P  # h chunks (4)
DC = D // P  # d chunks (4)
F32 = mybir.dt.float32
BF16 = mybir.dt.bfloat16
TANH = mybir.ActivationFunctionType.Tanh
EXP = mybir.ActivationFunctionType.Exp


def _body(ctx, tc, nc, q_d, k_d, wq_d, wk_d, wa_d, out_d, attn_d):
    singles = ctx.enter_context(tc.tile_pool(name="singles", bufs=1))
    keys_pool = ctx.enter_context(tc.tile_pool(name="keys", bufs=4))
    keysT_pool = ctx.enter_context(tc.tile_pool(name="keysT", bufs=3))

    # kick off batch 0's keys load first: it is the long pole for the first
    # compute wave and must not queue behind the small weight-setup DMAs
    keys_tiles = []
    for b in range(1):
        keys_sb = keys_pool.tile([P, ST, D], BF16, tag="keys")
        nc.gpsimd.dma_start(
            out=keys_sb, in_=k_d.ap()[b].rearrange("(st p) d -> p st d", p=P)
        )
        keys_tiles.append(keys_sb)

    # --- one-time setup: weights ---------------------------------------
    # natural-layout bf16 loads: [p, chunk, d] with p = h within chunk
    wk_nat = singles.tile([P, HC, D], BF16)
    nc.gpsimd.dma_start(out=wk_nat, in_=wk_d.ap().rearrange("(hc p) d -> p hc d", p=P))
    wq_nat = singles.tile([P, HC, D], BF16)
    nc.gpsimd.dma_start(out=wq_nat, in_=wq_d.ap().rearrange("(hc p) d -> p hc d", p=P))

    # block-transposed weights via one xbar call each:
    # wkT[p, j, l] = wk_nat[l, j*128+p]  ->  j = hc*DC + dc encodes (hc, dc);
    # lhsT chunk for (dc, hc) is wkT[:, hc*DC+dc, :] = WkT[d-chunk, h-chunk].
    wkT = singles.tile([P, HC * DC, P], BF16)
    nc.sync.dma_start(out=wkT, in_=wk_nat.rearrange("p hc d -> p (hc d)"), transpose=True)
    wqT = singles.tile([P, HC * DC, P], BF16)
    nc.sync.dma_start(out=wqT, in_=wq_nat.rearrange("p hc d -> p (hc d)"), transpose=True)

    # w_att as per-partition columns [p=h-in-chunk, hc]
    wa_col = singles.tile([P, HC], BF16)
    nc.gpsimd.dma_start(out=wa_col, in_=wa_d.ap().rearrange("(hc p) -> p hc", p=P))

    # query transposed [p=d-in-chunk, dc, b]
    qT = singles.tile([P, DC, BC], BF16)
    for dc in range(DC):
        nc.gpsimd.dma_start(
            out=qT[:, dc, :],
            in_=q_d.ap()[:, dc * P : (dc + 1) * P].rearrange("b p -> p b"),
        )

    # identity scalar for PE vector transposes
    ones11 = singles.tile([1, 1], F32)
    nc.vector.memset(ones11, 1.0)

    # Q = query @ Wq.T laid out [p=h-in-chunk, hc, b] (f32, used as ACT bias)
    q_sb = singles.tile([P, HC, BC], F32)
    with tc.tile_pool(name="ps_setup", bufs=2, space="PSUM") as psq:
        for hc in range(HC):
            pq = psq.tile([P, BC], F32)
            for dc in range(DC):
                nc.tensor.matmul(
                    pq,
                    lhsT=wqT[:, hc * DC + dc, :],
                    rhs=qT[:, dc, :],
                    start=(dc == 0),
                    stop=(dc == DC - 1),
                )
            nc.vector.tensor_copy(q_sb[:, hc, :], pq)

    # --- main loop pools ------------------------------------------------
    comb_pool = ctx.enter_context(tc.tile_pool(name="comb", bufs=6))
    p_pool = ctx.enter_context(tc.tile_pool(name="prow", bufs=2))
    small_pool = ctx.enter_context(tc.tile_pool(name="small", bufs=8))
    outbuf_pool = ctx.enter_context(tc.tile_pool(name="outbuf", bufs=2))
    ps_k = ctx.enter_context(tc.tile_pool(name="ps_k", bufs=2, space="PSUM"))  # 2x2 banks
    ps_sc = ctx.enter_context(tc.tile_pool(name="ps_sc", bufs=1, space="PSUM"))  # 2 banks
    ps_pcol = ctx.enter_context(tc.tile_pool(name="ps_pcol", bufs=1, space="PSUM"))
    ps_out = ctx.enter_context(tc.tile_pool(name="ps_out", bufs=1, space="PSUM"))

    for b in range(1, BC):
        keys_sb = keys_pool.tile([P, ST, D], BF16, tag="keys")
        nc.gpsimd.dma_start(
            out=keys_sb, in_=k_d.ap()[b].rearrange("(st p) d -> p st d", p=P)
        )
        keys_tiles.append(keys_sb)

    for b in range(BC):
        keys_sb = keys_tiles[b]
        # one batched xbar transpose: keysT[p, j, l] = keys_sb_flat[l, j*128+p]
        # with j = st*DC + dc; so [d-chunk dc | s-tile st] lives at mid j.
        keysT = keysT_pool.tile([P, ST * DC, P], BF16)
        nc.sync.dma_start(
            out=keysT, in_=keys_sb.rearrange("p st d -> p (st d)"), transpose=True
        )
        # view for the main matmul rhs: [p, dc, st, l]
        keysT_v = keysT.rearrange("p (st dc) l -> p dc st l", dc=DC)

        p_row = p_pool.tile([1, S], F32)
        denoms = small_pool.tile([1, 2], F32)
        pcol_bf = small_pool.tile([P, ST], BF16)
        ps_o = ps_out.tile([1, D], F32)

        for half in range(2):  # two s-block pairs of 1024 columns each
            # k-chunk matmuls + fused bias-tanh on [128, 1024] tiles
            comb_tiles = []
            for hc in range(HC):
                pk = ps_k.tile([P, 1024], F32)
                for sbi in range(2):
                    sb = half * 2 + sbi
                    for dc in range(DC):
                        nc.tensor.matmul(
                            pk[:, sbi * 512 : (sbi + 1) * 512],
                            lhsT=wkT[:, hc * DC + dc, :],
                            rhs=keysT_v[:, dc, sb * SB : (sb + 1) * SB, :],
                            start=(dc == 0),
                            stop=(dc == DC - 1),
                        )
                comb = comb_pool.tile([P, 1024], BF16)
                nc.scalar.activation(
                    comb, pk, TANH, bias=q_sb[:, hc, b : b + 1], scale=1.0
                )
                comb_tiles.append(comb)

            # scores[1, 1024] = sum_h w_att[h] * comb[h, s]
            psc = ps_sc.tile([1, 1024], F32)
            for sbi in range(2):
                for hc in range(HC):
                    nc.tensor.matmul(
                        psc[:, sbi * 512 : (sbi + 1) * 512],
                        lhsT=wa_col[:, hc : hc + 1],
                        rhs=comb_tiles[hc][:, sbi * 512 : (sbi + 1) * 512],
                        start=(hc == 0),
                        stop=(hc == HC - 1),
                    )
            # p = exp(scores) for 1024 cols, plus partial denominator
            hcol = slice(half * 1024, (half + 1) * 1024)
            nc.scalar.activation(
                p_row[:, hcol], psc, EXP, accum_out=denoms[:, half : half + 1]
            )
            # p columns for the attn@keys matmul: PE transpose of p chunks
            ppc = ps_pcol.tile([P, 8], F32)
            for j in range(8):
                st = half * 8 + j
                nc.tensor.transpose(
                    ppc[:, j : j + 1], p_row[0:1, st * P : (st + 1) * P], ones11
                )
            nc.vector.tensor_copy(pcol_bf[:, half * 8 : (half + 1) * 8], ppc)

            # out_un[1, d] += p[st].T @ keys[st]
            for j in range(8):
                st = half * 8 + j
                nc.tensor.matmul(
                    ps_o,
                    lhsT=pcol_bf[:, st : st + 1],
                    rhs=keys_sb[:, st, :],
                    start=(st == 0),
                    stop=(st == ST - 1),
                )

        # denominator + normalization
        dsum = small_pool.tile([1, 1], F32)
        nc.vector.tensor_reduce(
            dsum, denoms, axis=mybir.AxisListType.X, op=mybir.AluOpType.add
        )
        recip = small_pool.tile([1, 1], F32)
        nc.vector.reciprocal(recip, dsum)

        outrow = outbuf_pool.tile([1, D], F32)
        nc.vector.tensor_scalar_mul(outrow, ps_o, recip)
        nc.sync.dma_start(out=out_d.ap()[b : b + 1, :], in_=outrow)

        attn_row = outbuf_pool.tile([1, S], F32)
        nc.vector.tensor_scalar_mul(attn_row, p_row, recip)
        nc.sync.dma_start(out=attn_d.ap()[b : b + 1, :], in_=attn_row)


def _build():
    from contextlib import ExitStack

    nc = bacc.Bacc("TRN2", target_bir_lowering=False, debug=False, num_devices=NCORES)
    q_d = nc.dram_tensor("query", (BC, D), F32, kind="ExternalInput")
    k_d = nc.dram_tensor("keys", (BC, S, D), F32, kind="ExternalInput")
    wq_d = nc.dram_tensor("wq", (H, D), F32, kind="ExternalInput")
    wk_d = nc.dram_tensor("wk", (H, D), F32, kind="ExternalInput")
    wa_d = nc.dram_tensor("watt", (H,), F32, kind="ExternalInput")
    out_d = nc.dram_tensor("out", (BC, D), F32, kind="ExternalOutput")
    attn_d = nc.dram_tensor("attn", (BC, S), F32, kind="ExternalOutput")

    with tile.TileContext(nc) as tc:
        with ExitStack() as ctx:
            _body(ctx, tc, nc, q_d, k_d, wq_d, wk_d, wa_d, out_d, attn_d)
    nc.compile()
    return nc


_NC_CACHE = None


def _get_nc():
    global _NC_CACHE
    if _NC_CACHE is None:
        _NC_CACHE = _build()
    return _NC_CACHE


def kernel(query, keys, Wq, Wk, w_att, _trace=False, _tmpdir=None):
    nc = _get_nc()
    query = np.ascontiguousarray(query, dtype=np.float32)
    keys = np.ascontiguousarray(keys, dtype=np.float32)
    Wq = np.ascontiguousarray(Wq, dtype=np.float32)
    Wk = np.ascontiguousarray(Wk, dtype=np.float32)
    w_att = np.ascontiguousarray(w_att, dtype=np.float32)

    in_maps = [
        {
            "query": query[c * BC : (c + 1) * BC],
            "keys": keys[c * BC : (c + 1) * BC],
            "wq": Wq,
            "wk": Wk,
            "watt": w_att,
        }
        for c in range(NCORES)
    ]
    res = bass_utils.run_bass_kernel_spmd(
        nc, in_maps, core_ids=list(range(NCORES)), trace=_trace, tmpdir=_tmpdir
    )
    out = np.concatenate([res.results[c]["out"] for c in range(NCORES)], axis=0)
    attn = np.concatenate([res.results[c]["attn"] for c in range(NCORES)], axis=0)
    if _trace:
        return (out, attn), res
    return out, attn
